# revision 1
# baseline (speedup 1.0000x reference)
"""Block-sparse local+vertical-stride causal attention for Trainium2 (Bass/Tile).

Problem: B=1, S=2048, H=32, D=128, sparse_block=64, local_blocks=16,
vert_stride=8, head_sliding_step=1. Mask per head h:
  causal(q,k) AND ( (q_blk - k_blk < 16) OR ((k_blk + h + 1) % 8 == 0) )

Sharding: 8 cores; core c computes heads {c, c+8, c+16, c+24}. All four share
the same vertical-stride residue r = (7 - c) % 8, so a single compiled SPMD
program works for every core with per-core *data* (masks + pre-gathered
vertical K/V blocks); the code is identical on all cores.

Device algorithm per (head, q-tile of 256 tokens):
  - scores computed transposed S_T[k, q] = K^T-stationary.T @ Q^T-moving on
    the PE in float32r (full rate at moving dim >= 256)
  - window = up to 10 k-tiles of 128 tokens (16 local blocks + 4 diag-region
    blocks) + 1 pre-gathered vertical tile (blocks {r, r+8})
  - one exp per PSUM chunk on ScalarE (scale = D^-0.5 folded in); no max
    subtraction needed (scores bounded ~20 -> exp well within fp32 range)
  - 0/1 multiplicative masks on VectorE for the q-dependent boundary tiles,
    the token-causal diagonal region and the vertical tile
  - PV: out_T[d, q] += V-stationary.T @ exp_S_T-moving  (no P transpose)
  - denominator via ones-column matmul riding the same PSUM bank as PV,
    reciprocal on DVE, partition-broadcast DMA, normalization folded into the
    PSUM->SBUF copy of out_T
Host reassembles heads and flips the per-head [d, q] layout to [q, d].
"""

import sys
import types

import numpy as np

# ----------------------------------------------------------------------------
# problem constants (hardcoded per contract; kernel.py must be self-contained)
B, S, H, D = 1, 2048, 32, 128
BLOCK = 64
LOCAL = 16
VERT = 8
NCORES = 8
HPC = H // NCORES  # heads per core (4)
QT = 256  # q tokens per window
NT = S // QT  # 8 windows per head
NKT = S // 128  # 16 k-tiles of 128 tokens per head
SCALE = float(D) ** -0.5

# matmul input dtype: "float32r" (full-rate fp32 tensor-engine mode),
# "bfloat16", or "float32" (4x slower, exact)
MM_DT = "float32r"


def _install_ntff_shim():
    """bass_utils wants antenv.axon_hooks (absent in this image); provide it,
    backed by the ctypes NTFF profiler from trn_agent_boot when available."""
    if "antenv.axon_hooks" in sys.modules:
        return
    hook = None
    try:
        from trn_agent_boot.trn_boot import _ntff_profile_via_ctypes

        hook = _ntff_profile_via_ctypes("/opt/axon/libaxon_pjrt.so")
    except Exception:
        hook = None
    m = types.ModuleType("antenv.axon_hooks")
    m.get_axon_ntff_profile_hook = lambda: hook
    m.set_axon_ntff_profile_hook = lambda h: None
    sys.modules["antenv.axon_hooks"] = m


def _i_min(t):
    """First window k-tile index for q-tile t. Tile i covers sparse blocks
    (4t-16+2i, 4t-15+2i), i in [i_min, 10); i in {8, 9} is the diag region."""
    return max(0, 8 - 2 * t)


def _has_vert(t):
    return t >= 5


_NC = None


def _build():
    """Build + compile the per-core Bass program (one NEFF, all cores)."""
    global _NC
    if _NC is not None:
        return _NC
    import concourse.mybir as mybir
    import concourse.tile as tile
    from concourse import bacc

    f32 = mybir.dt.float32
    mdt = getattr(mybir.dt, MM_DT)

    nc = bacc.Bacc("TRN2", target_bir_lowering=False, debug=False,
                   num_devices=NCORES)

    qt_d = nc.dram_tensor("qt", [HPC, D, S], mdt, kind="ExternalInput")
    kt_d = nc.dram_tensor("kt", [HPC, D, S], mdt, kind="ExternalInput")
    v_d = nc.dram_tensor("v", [HPC, S, D], mdt, kind="ExternalInput")
    ktv_d = nc.dram_tensor("ktv", [HPC, D, 128], mdt, kind="ExternalInput")
    vv_d = nc.dram_tensor("vv", [HPC, 128, D], mdt, kind="ExternalInput")
    wm_d = nc.dram_tensor("wmask", [NT, 2, 128, QT], f32, kind="ExternalInput")
    dm_d = nc.dram_tensor("dmask", [2, 128, QT], f32, kind="ExternalInput")
    vm_d = nc.dram_tensor("vmask", [NT, 128], f32, kind="ExternalInput")
    o_d = nc.dram_tensor("o", [HPC, D, S], f32, kind="ExternalOutput")
    den_d = nc.dram_tensor("den", [HPC, 1, S], f32, kind="ExternalOutput")

    with tile.TileContext(nc) as tc:
        with (
            tc.tile_pool(name="consts", bufs=1) as consts,
            tc.tile_pool(name="io", bufs=2) as io,
            tc.tile_pool(name="exps", bufs=2) as exps,
            tc.tile_pool(name="small", bufs=3) as small,
            tc.tile_pool(name="psA", bufs=1, space="PSUM") as psA,
            tc.tile_pool(name="psB", bufs=1, space="PSUM") as psB,
            tc.tile_pool(name="psPV", bufs=2, space="PSUM") as psPV,
        ):
            dmask = consts.tile([128, 2, QT], f32)
            nc.gpsimd.dma_start(out=dmask, in_=dm_d.ap().rearrange("i p q -> p i q"))
            vmask = consts.tile([128, NT], f32)
            nc.gpsimd.dma_start(out=vmask, in_=vm_d.ap().rearrange("t p -> p t"))
            wmask = consts.tile([128, NT, 2, QT], f32)
            ones_f32 = consts.tile([128, 1], f32)
            nc.vector.memset(ones_f32, 1.0)
            ones_col = consts.tile([128, 1], mdt)
            nc.vector.tensor_copy(out=ones_col, in_=ones_f32)
            ones_row_f32 = consts.tile([1, 128], f32)
            nc.vector.memset(ones_row_f32, 1.0)
            ones_row = consts.tile([1, 128], mdt)
            nc.vector.tensor_copy(out=ones_row, in_=ones_row_f32)

            for h in range(HPC):
                qt_sb = io.tile([128, S], mdt, tag="qt")
                kt_sb = io.tile([128, S], mdt, tag="kt")
                v_sb = io.tile([128, NKT, 128], mdt, tag="v")
                vre = v_d.ap()[h].rearrange("(j p) d -> p j d", p=128)
                # first window's exact inputs as dedicated head-of-queue DMAs
                nc.sync.dma_start(out=kt_sb[:, 0:512], in_=kt_d.ap()[h][:, 0:512])
                nc.sync.dma_start(out=qt_sb[:, 0:256], in_=qt_d.ap()[h][:, 0:256])
                nc.sync.dma_start(out=v_sb[:, 0:4, :], in_=vre[:, 0:4, :])
                nc.sync.dma_start(out=kt_sb[:, 512:1024],
                                  in_=kt_d.ap()[h][:, 512:1024])
                nc.sync.dma_start(out=qt_sb[:, 256:1024],
                                  in_=qt_d.ap()[h][:, 256:1024])
                for ch in range(2, 4):
                    cs = ch * (S // 4)
                    nc.sync.dma_start(out=kt_sb[:, cs:cs + S // 4],
                                      in_=kt_d.ap()[h][:, cs:cs + S // 4])
                    nc.sync.dma_start(out=qt_sb[:, cs:cs + S // 4],
                                      in_=qt_d.ap()[h][:, cs:cs + S // 4])
                js = NKT // 4
                nc.sync.dma_start(out=v_sb[:, js:, :], in_=vre[:, js:, :])
                ktv_sb = io.tile([128, 128], mdt, tag="ktv")
                nc.gpsimd.dma_start(out=ktv_sb, in_=ktv_d.ap()[h])
                vv_sb = io.tile([128, 128], mdt, tag="vv")
                nc.gpsimd.dma_start(out=vv_sb, in_=vv_d.ap()[h])
                if h == 0:
                    nc.gpsimd.dma_start(
                        out=wmask, in_=wm_d.ap().rearrange("t i p q -> p t i q")
                    )

                outT = io.tile([128, S], f32, tag="outT")
                den_sb = io.tile([1, S], f32, tag="den_sb")

                def emit_scores(t):
                    """S_T matmuls + exp + masks for window t. Returns the
                    (exp-slice, v-slice) matmul source list for the PV stage."""
                    im = _i_min(t)
                    nA = 8 - im
                    vert = _has_vert(t)
                    nB = 2 + (1 if vert else 0)
                    q_sl = qt_sb[:, t * QT:(t + 1) * QT]

                    expA = None
                    if nA:
                        sA = psA.tile([128, nA * QT], f32, tag="sA")
                        for a in range(nA):
                            i = im + a
                            toff = 256 * t - 1024 + 128 * i
                            nc.tensor.matmul(
                                sA[:, a * QT:(a + 1) * QT],
                                kt_sb[:, toff:toff + 128], q_sl,
                                start=True, stop=True,
                            )
                    sB = psB.tile([128, nB * QT], f32, tag="sB")
                    for b in range(2):
                        toff = 256 * t + 128 * b
                        nc.tensor.matmul(
                            sB[:, b * QT:(b + 1) * QT],
                            kt_sb[:, toff:toff + 128], q_sl,
                            start=True, stop=True,
                        )
                    if vert:
                        nc.tensor.matmul(
                            sB[:, 2 * QT:3 * QT], ktv_sb, q_sl,
                            start=True, stop=True,
                        )

                    if nA:
                        expA = exps.tile([128, nA * QT], mdt, tag="expA")
                        nc.scalar.activation(
                            expA, sA, mybir.ActivationFunctionType.Exp,
                            scale=SCALE,
                        )
                        if t >= 4:
                            for i in range(2):
                                nc.vector.tensor_mul(
                                    expA[:, i * QT:(i + 1) * QT],
                                    expA[:, i * QT:(i + 1) * QT],
                                    wmask[:, t, i, :],
                                )
                    expB = exps.tile([128, nB * QT], mdt, tag="expB")
                    nc.scalar.activation(
                        expB, sB, mybir.ActivationFunctionType.Exp, scale=SCALE
                    )
                    for b in range(2):
                        nc.vector.tensor_mul(
                            expB[:, b * QT:(b + 1) * QT],
                            expB[:, b * QT:(b + 1) * QT],
                            dmask[:, b, :],
                        )
                    if vert:
                        nc.vector.tensor_scalar_mul(
                            out=expB[:, 2 * QT:3 * QT],
                            in0=expB[:, 2 * QT:3 * QT],
                            scalar1=vmask[:, t:t + 1],
                        )

                    srcs = []
                    for a in range(nA):
                        i = im + a
                        srcs.append((expA[:, a * QT:(a + 1) * QT],
                                     v_sb[:, 2 * t - 8 + i, :]))
                    for b in range(2):
                        srcs.append((expB[:, b * QT:(b + 1) * QT],
                                     v_sb[:, 2 * t + b, :]))
                    if vert:
                        srcs.append((expB[:, 2 * QT:3 * QT], vv_sb))
                    return srcs

                def emit_pv(t, srcs):
                    """PV + denominator matmul chains (NOT interleaved: the PE
                    needs contiguous start/stop groups per PSUM region), then
                    stage out_T and den to SBUF."""
                    pv = psPV.tile([128, 512], f32, tag="pv")
                    n_mm = len(srcs)
                    for k, (e_sl, v_sl) in enumerate(srcs):
                        nc.tensor.matmul(
                            pv[:, 0:QT], v_sl, e_sl,
                            start=(k == 0), stop=(k == n_mm - 1),
                        )
                    for k, (e_sl, _) in enumerate(srcs):
                        nc.tensor.matmul(
                            pv[0:1, QT:2 * QT], ones_col, e_sl,
                            start=(k == 0), stop=(k == n_mm - 1),
                        )
                    # stage unnormalized out_T and den to SBUF; the final
                    # divide happens host-side during the un-shard/gather
                    # (device reciprocal measured ~9.5 cycles/column on DVE)
                    nc.vector.tensor_copy(
                        out=den_sb[0:1, t * QT:(t + 1) * QT],
                        in_=pv[0:1, QT:2 * QT],
                    )
                    nc.vector.tensor_copy(
                        out=outT[:, t * QT:(t + 1) * QT], in_=pv[:, 0:QT]
                    )
                    nc.sync.dma_start(
                        out=o_d.ap()[h][:, t * QT:(t + 1) * QT],
                        in_=outT[:, t * QT:(t + 1) * QT],
                    )

                # software-pipelined emission: scores(t+1) lands before pv(t)
                # so the PE always has matmul work while ACT/DVE process t
                prev = None
                for t in range(NT):
                    srcs = emit_scores(t)
                    if prev is not None:
                        emit_pv(prev[0], prev[1])
                    prev = (t, srcs)
                emit_pv(prev[0], prev[1])

                nc.sync.dma_start(out=den_d.ap()[h], in_=den_sb)

    nc.compile()
    _NC = nc
    return nc


def _host_prep(query, key, value, core):
    """Per-core input dict. query/key/value: [B, S, H, D] float32 (full)."""
    heads = [core + NCORES * i for i in range(HPC)]
    r = (7 - core) % VERT
    q = query[0][:, heads, :]  # [S, 4, D]
    k = key[0][:, heads, :]
    v = value[0][:, heads, :]
    qt = np.ascontiguousarray(q.transpose(1, 2, 0))  # [4, D, S]
    kt = np.ascontiguousarray(k.transpose(1, 2, 0))
    vn = np.ascontiguousarray(v.transpose(1, 0, 2))  # [4, S, D]
    # vertical gather: k-tokens of blocks {r, r+8}
    vtok = np.concatenate([
        np.arange(r * BLOCK, (r + 1) * BLOCK),
        np.arange((r + 8) * BLOCK, (r + 9) * BLOCK),
    ])
    ktv = np.ascontiguousarray(kt[:, :, vtok])  # [4, D, 128]
    vv = np.ascontiguousarray(vn[:, vtok, :])  # [4, 128, D]

    # masks
    wm = np.ones((NT, 2, 128, QT), dtype=np.float32)
    for t in range(4, NT):
        for i in range(2):
            for ph in range(2):  # partition half -> block
                kb = 4 * t - 16 + 2 * i + ph
                for qb in range(4):
                    act = (qb + 4 * t - kb < LOCAL) or (kb % VERT == r)
                    wm[t, i, ph * 64:(ph + 1) * 64, qb * 64:(qb + 1) * 64] = (
                        1.0 if act else 0.0
                    )
    dm = np.zeros((2, 128, QT), dtype=np.float32)
    for i in range(2):
        for p in range(128):
            k_rel = 128 * i + p
            dm[i, p, k_rel:] = 1.0
    vm = np.zeros((NT, 128), dtype=np.float32)
    for t in range(NT):
        for ph in range(2):
            kb = r + 8 * ph
            if kb < 4 * t - 16:
                vm[t, ph * 64:(ph + 1) * 64] = 1.0

    if MM_DT == "bfloat16":
        import ml_dtypes
        bf16 = ml_dtypes.bfloat16
        qt, kt, vn = qt.astype(bf16), kt.astype(bf16), vn.astype(bf16)
        ktv, vv = ktv.astype(bf16), vv.astype(bf16)
    return {
        "qt": qt, "kt": kt, "v": vn, "ktv": ktv, "vv": vv,
        "wmask": wm, "dmask": dm, "vmask": vm,
    }


def kernel(query, key, value, _trace=False, _tmpdir=None):
    """Full-input entry point: [1, 2048, 32, 128] f32 each -> same shape."""
    _install_ntff_shim()
    from concourse.bass_utils import run_bass_kernel_spmd

    query = np.asarray(query, dtype=np.float32)
    key = np.asarray(key, dtype=np.float32)
    value = np.asarray(value, dtype=np.float32)

    nc = _build()
    in_maps = [_host_prep(query, key, value, c) for c in range(NCORES)]
    res = run_bass_kernel_spmd(
        nc, in_maps, core_ids=list(range(NCORES)),
        trace=_trace, tmpdir=_tmpdir,
    )
    out = np.empty((B, S, H, D), dtype=np.float32)
    for c in range(NCORES):
        o = res.results[c]["o"] / res.results[c]["den"]  # [4, D, S] / [4, 1, S]
        for i in range(HPC):
            out[0, :, c + NCORES * i, :] = o[i].T
    kernel.last_result = res
    return out



# revision 6
# speedup vs baseline: 1.1551x; 1.1551x over previous
"""Block-sparse local+vertical-stride causal attention for Trainium2 (Bass/Tile).

Problem: B=1, S=2048, H=32, D=128, sparse_block=64, local_blocks=16,
vert_stride=8, head_sliding_step=1. Mask per head h:
  causal(q,k) AND ( (q_blk - k_blk < 16) OR ((k_blk + h + 1) % 8 == 0) )

Sharding: 8 cores; core c computes heads {c, c+8, c+16, c+24}. All four share
the same vertical-stride residue r = (7 - c) % 8, so a single compiled SPMD
program works for every core with per-core *data* (small masks + pre-gathered
vertical K/V blocks); the code is identical on all cores.

Device algorithm per (head, q-window of 256 tokens = 4 sparse blocks):
  scores S_T[k, q] on the PE in bf16 (same PE rate as fp32r, half the
  DMA/SBUF), tiled to track the sparse structure exactly:
    t >= 4:  L0 (k-blocks 4t-16,4t-15; only qb0 -> 64 q-cols)
             L1 (4t-14,4t-13; qb0..2 -> 192 q-cols)
             L2..L7 (4t-12..4t-1; full 256)
             D0 (4t,4t+1; full 256, causal triangle on chunk 0)
             D1 (4t+2,4t+3; 128 q-cols, causal triangle)
             vert (pre-gathered blocks {r, r+8}; full 256; per-partition
                   0/1 vmask selects blocks with kb < 4t-12)
    t < 4:   2t full L-tiles from block 0 (all-local) + D0 + D1
  One exp per PSUM region on ScalarE (scale folded in; bf16 out). Masking:
    - wm_small [128, 256] multiplicative mask on the L0/L1 region (dead
      sub-blocks + vert/local double-count resolution) - per-window data
    - constant aligned [128,128] causal triangle multiplied into the two
      diag chunks
    - vmask per-partition tensor_scalar on the vert tile
  PV: out_T[d, q] += V.T-stationary @ exp-moving; denominator via a
  ones-column matmul chain riding the same PSUM bank. Host divides.
Host reassembles heads and flips the per-head [d, q] layout to [q, d].
"""

import sys
import types

import numpy as np

# ----------------------------------------------------------------------------
# problem constants (hardcoded per contract; kernel.py must be self-contained)
B, S, H, D = 1, 2048, 32, 128
BLOCK = 64
LOCAL = 16
VERT = 8
NCORES = 8
HPC = H // NCORES  # heads per core (4)
QT = 256  # q tokens per window
NT = S // QT  # 8 windows per head
NKT = S // 128  # 16 k-tiles of 128 tokens per head
SCALE = float(D) ** -0.5

MM_DT = "bfloat16"  # matmul input dtype ("bfloat16" or "float32r")

# psA column layout for t >= 4: [L0 64 | L1 192 | L2..L7 6x256] = 1792
PSA_COLS = 1792
# psB column layout: [D0 256 | D1 128 | pad 128 | vert 256] = 768
# (pad keeps the vert region inside a single PSUM bank: cols 512:768)
PSB_COLS = 768
VOFF = 512


def _install_ntff_shim():
    """bass_utils wants antenv.axon_hooks (absent in this image); provide it,
    backed by the ctypes NTFF profiler from trn_agent_boot when available."""
    if "antenv.axon_hooks" in sys.modules:
        return
    hook = None
    try:
        from trn_agent_boot.trn_boot import _ntff_profile_via_ctypes

        hook = _ntff_profile_via_ctypes("/opt/axon/libaxon_pjrt.so")
    except Exception:
        hook = None
    m = types.ModuleType("antenv.axon_hooks")
    m.get_axon_ntff_profile_hook = lambda: hook
    m.set_axon_ntff_profile_hook = lambda h: None
    sys.modules["antenv.axon_hooks"] = m


_NC = None


def _build():
    """Build + compile the per-core Bass program (one NEFF, all cores)."""
    global _NC
    if _NC is not None:
        return _NC
    import concourse.mybir as mybir
    import concourse.tile as tile
    from concourse import bacc

    f32 = mybir.dt.float32
    mdt = getattr(mybir.dt, MM_DT)

    nc = bacc.Bacc("TRN2", target_bir_lowering=False, debug=False,
                   num_devices=NCORES)

    qt_d = nc.dram_tensor("qt", [HPC, D, S], mdt, kind="ExternalInput")
    kt_d = nc.dram_tensor("kt", [HPC, D, S], mdt, kind="ExternalInput")
    v_d = nc.dram_tensor("v", [HPC, S, D], mdt, kind="ExternalInput")
    ktv_d = nc.dram_tensor("ktv", [HPC, D, 128], mdt, kind="ExternalInput")
    vv_d = nc.dram_tensor("vv", [HPC, 128, D], mdt, kind="ExternalInput")
    wm_d = nc.dram_tensor("wm", [NT, 128, 256], mdt, kind="ExternalInput")
    tri_d = nc.dram_tensor("tri", [128, 128], mdt, kind="ExternalInput")
    vm_d = nc.dram_tensor("vmask", [NT, 128], f32, kind="ExternalInput")
    o_d = nc.dram_tensor("o", [HPC, D, S], f32, kind="ExternalOutput")
    den_d = nc.dram_tensor("den", [HPC, 1, S], f32, kind="ExternalOutput")

    with tile.TileContext(nc) as tc:
        with (
            tc.tile_pool(name="consts", bufs=1) as consts,
            tc.tile_pool(name="io", bufs=2) as io,
            tc.tile_pool(name="exps", bufs=2) as exps,
            tc.tile_pool(name="psA", bufs=1, space="PSUM") as psA,
            tc.tile_pool(name="psB", bufs=1, space="PSUM") as psB,
            tc.tile_pool(name="psPV", bufs=2, space="PSUM") as psPV,
        ):
            tri = consts.tile([128, 128], mdt)
            nc.gpsimd.dma_start(out=tri, in_=tri_d.ap())
            vmask = consts.tile([128, NT], f32)
            nc.gpsimd.dma_start(out=vmask, in_=vm_d.ap().rearrange("t p -> p t"))
            wmask = consts.tile([128, NT, 256], mdt)
            nc.gpsimd.dma_start(out=wmask,
                                in_=wm_d.ap().rearrange("t p q -> p t q"))
            ones_f32 = consts.tile([128, 1], f32)
            nc.vector.memset(ones_f32, 1.0)
            ones_col = consts.tile([128, 1], mdt)
            nc.vector.tensor_copy(out=ones_col, in_=ones_f32)

            for h in range(HPC):
                qt_sb = io.tile([128, S], mdt, tag="qt")
                kt_sb = io.tile([128, S], mdt, tag="kt")
                v_sb = io.tile([128, NKT, 128], mdt, tag="v")
                vre = v_d.ap()[h].rearrange("(j p) d -> p j d", p=128)
                # first window's exact inputs as dedicated head-of-queue DMAs
                nc.sync.dma_start(out=kt_sb[:, 0:512], in_=kt_d.ap()[h][:, 0:512])
                nc.sync.dma_start(out=qt_sb[:, 0:256], in_=qt_d.ap()[h][:, 0:256])
                nc.sync.dma_start(out=v_sb[:, 0:4, :], in_=vre[:, 0:4, :])
                nc.sync.dma_start(out=kt_sb[:, 512:1024],
                                  in_=kt_d.ap()[h][:, 512:1024])
                nc.sync.dma_start(out=qt_sb[:, 256:1024],
                                  in_=qt_d.ap()[h][:, 256:1024])
                for ch in range(2, 4):
                    cs = ch * (S // 4)
                    nc.sync.dma_start(out=kt_sb[:, cs:cs + S // 4],
                                      in_=kt_d.ap()[h][:, cs:cs + S // 4])
                    nc.sync.dma_start(out=qt_sb[:, cs:cs + S // 4],
                                      in_=qt_d.ap()[h][:, cs:cs + S // 4])
                js = NKT // 4
                nc.sync.dma_start(out=v_sb[:, js:, :], in_=vre[:, js:, :])
                ktv_sb = io.tile([128, 128], mdt, tag="ktv")
                nc.gpsimd.dma_start(out=ktv_sb, in_=ktv_d.ap()[h])
                vv_sb = io.tile([128, 128], mdt, tag="vv")
                nc.gpsimd.dma_start(out=vv_sb, in_=vv_d.ap()[h])

                outT = io.tile([128, S], f32, tag="outT")
                den_sb = io.tile([1, S], f32, tag="den_sb")

                def emit_scores(t):
                    """Scores matmuls + exp + masks for window t. Returns the
                    (exp-slice, v-slice) list for the PV/den stage."""
                    q_sl = qt_sb[:, t * QT:(t + 1) * QT]
                    srcs = []

                    expA = None
                    if t >= 4:
                        sA = psA.tile([128, PSA_COLS], f32, tag="sA")
                        # L0: k-blocks 4t-16,4t-15 (k-tile 2t-8), qb0 only
                        nc.tensor.matmul(
                            sA[:, 0:64],
                            kt_sb[:, (2 * t - 8) * 128:(2 * t - 7) * 128],
                            q_sl[:, 0:64], start=True, stop=True,
                        )
                        # L1: k-blocks 4t-14,4t-13 (k-tile 2t-7), qb0..2
                        nc.tensor.matmul(
                            sA[:, 64:256],
                            kt_sb[:, (2 * t - 7) * 128:(2 * t - 6) * 128],
                            q_sl[:, 0:192], start=True, stop=True,
                        )
                        # L2..L7: k-tiles 2t-6 .. 2t-1, full 256
                        for j in range(6):
                            kt_i = 2 * t - 6 + j
                            nc.tensor.matmul(
                                sA[:, 256 * (j + 1):256 * (j + 2)],
                                kt_sb[:, kt_i * 128:(kt_i + 1) * 128],
                                q_sl, start=True, stop=True,
                            )
                        expA = exps.tile([128, PSA_COLS], mdt, tag="expA")
                        nc.scalar.activation(
                            expA, sA, mybir.ActivationFunctionType.Exp,
                            scale=SCALE,
                        )
                        # data-driven mask on the L0/L1 region (dead blocks,
                        # vert/local double-count)
                        nc.vector.tensor_mul(
                            expA[:, 0:256], expA[:, 0:256], wmask[:, t, :]
                        )
                        # full-width tiles first: the first matmul of the
                        # PV/den accumulation chains must cover the whole
                        # [0:256] q range so has_written is set everywhere
                        for j in range(6):
                            srcs.append((expA[:, 256 * (j + 1):256 * (j + 2)],
                                         v_sb[:, 2 * t - 6 + j, :]))
                        srcs.append((expA[:, 0:64], v_sb[:, 2 * t - 8, :]))
                        srcs.append((expA[:, 64:256], v_sb[:, 2 * t - 7, :]))
                    elif t >= 1:
                        sA = psA.tile([128, 512 * t], f32, tag="sA")
                        for j in range(2 * t):
                            nc.tensor.matmul(
                                sA[:, 256 * j:256 * (j + 1)],
                                kt_sb[:, j * 128:(j + 1) * 128],
                                q_sl, start=True, stop=True,
                            )
                        expA = exps.tile([128, 512 * t], mdt, tag="expA")
                        nc.scalar.activation(
                            expA, sA, mybir.ActivationFunctionType.Exp,
                            scale=SCALE,
                        )
                        for j in range(2 * t):
                            srcs.append((expA[:, 256 * j:256 * (j + 1)],
                                         v_sb[:, j, :]))

                    # B region: D0 full, D1 chunk1 only, vert (t>=4)
                    vert = t >= 4
                    sB = psB.tile([128, PSB_COLS], f32, tag="sB")
                    nc.tensor.matmul(
                        sB[:, 0:256],
                        kt_sb[:, (2 * t) * 128:(2 * t + 1) * 128],
                        q_sl, start=True, stop=True,
                    )
                    nc.tensor.matmul(
                        sB[:, 256:384],
                        kt_sb[:, (2 * t + 1) * 128:(2 * t + 2) * 128],
                        q_sl[:, 128:256], start=True, stop=True,
                    )
                    if vert:
                        nc.tensor.matmul(
                            sB[:, VOFF:VOFF + 256], ktv_sb, q_sl,
                            start=True, stop=True,
                        )
                    expB = exps.tile([128, PSB_COLS], mdt, tag="expB")
                    ncols = VOFF + 256 if vert else 384
                    nc.scalar.activation(
                        expB[:, 0:ncols], sB[:, 0:ncols],
                        mybir.ActivationFunctionType.Exp, scale=SCALE,
                    )
                    # causal triangles on the two diag chunks
                    nc.vector.tensor_mul(expB[:, 0:128], expB[:, 0:128], tri)
                    nc.vector.tensor_mul(expB[:, 256:384], expB[:, 256:384],
                                         tri)
                    if vert:
                        nc.vector.tensor_scalar_mul(
                            out=expB[:, VOFF:VOFF + 256],
                            in0=expB[:, VOFF:VOFF + 256],
                            scalar1=vmask[:, t:t + 1],
                        )
                    srcs.append((expB[:, 0:256], v_sb[:, 2 * t, :]))
                    srcs.append((expB[:, 256:384], v_sb[:, 2 * t + 1, :]))
                    if vert:
                        srcs.append((expB[:, VOFF:VOFF + 256], vv_sb))
                    return srcs

                def emit_pv(t, srcs):
                    """PV + denominator matmul chains, then stage out_T and
                    den to SBUF and DMA out_T."""
                    pv = psPV.tile([128, 512], f32, tag="pv")
                    n_mm = len(srcs)
                    # q-column range of each exp slice within the window:
                    # L0 -> [0:64], L1 -> [0:192], D1 -> [128:256], else full
                    def qrange(e_sl):
                        n = e_sl.shape[-1]
                        if n == 64:
                            return 0, 64
                        if n == 192:
                            return 0, 192
                        if n == 128:
                            return 128, 256
                        return 0, 256

                    for k, (e_sl, v_sl) in enumerate(srcs):
                        lo, hi = qrange(e_sl)
                        nc.tensor.matmul(
                            pv[:, lo:hi], v_sl, e_sl,
                            start=(k == 0), stop=(k == n_mm - 1),
                        )
                    for k, (e_sl, _) in enumerate(srcs):
                        lo, hi = qrange(e_sl)
                        nc.tensor.matmul(
                            pv[0:1, 256 + lo:256 + hi], ones_col, e_sl,
                            start=(k == 0), stop=(k == n_mm - 1),
                        )
                    nc.vector.tensor_copy(
                        out=den_sb[0:1, t * QT:(t + 1) * QT],
                        in_=pv[0:1, 256:512],
                    )
                    nc.vector.tensor_copy(
                        out=outT[:, t * QT:(t + 1) * QT], in_=pv[:, 0:256]
                    )
                    nc.sync.dma_start(
                        out=o_d.ap()[h][:, t * QT:(t + 1) * QT],
                        in_=outT[:, t * QT:(t + 1) * QT],
                    )

                # software-pipelined emission: scores(t+1) lands before pv(t)
                # so the PE always has matmul work while ACT/DVE process t
                prev = None
                for t in range(NT):
                    srcs = emit_scores(t)
                    if prev is not None:
                        emit_pv(prev[0], prev[1])
                    prev = (t, srcs)
                emit_pv(prev[0], prev[1])

                nc.sync.dma_start(out=den_d.ap()[h], in_=den_sb)

    nc.compile()
    _NC = nc
    return nc


def _host_masks(r):
    """Per-core mask data: wm [NT,128,256], vmask [NT,128], tri [128,128]."""
    # wm: multiplies expA[:, 0:256] (L0 cols 0:64 + L1 cols 64:256), t >= 4.
    # partition p: L0 -> block 4t-16 (p<64) / 4t-15 (p>=64)
    #              L1 -> block 4t-14 (p<64) / 4t-13 (p>=64)
    wm = np.ones((NT, 128, 256), dtype=np.float32)
    for t in range(4, NT):
        bl0, bl1 = 4 * t - 16, 4 * t - 15
        bl2, bl3 = 4 * t - 14, 4 * t - 13
        vc = lambda kb: kb % VERT == r  # vert tile covers kb (kb < 4t-12 here)
        # L0 region (cols 0:64 = qb0)
        wm[t, 0:64, 0:64] = 0.0  # block 4t-16 never local
        if vc(bl1):
            wm[t, 64:128, 0:64] = 0.0  # vert tile owns block 4t-15
        # L1 region (cols 64:256 = qb0..2)
        if vc(bl2):
            wm[t, 0:64, 64:256] = 0.0
        else:
            wm[t, 0:64, 64 + 128:256] = 0.0  # qb2 not local for 4t-14
        if vc(bl3):
            wm[t, 64:128, 64:256] = 0.0
        # block 4t-13 local for qb0..2 (all L1 cols) when not vert-covered
    # vmask: vert tile partitions p -> kb = r (p<64) / r+8 (p>=64);
    # active iff kb < 4t-12
    vm = np.zeros((NT, 128), dtype=np.float32)
    for t in range(4, NT):
        if r < 4 * t - 12:
            vm[t, 0:64] = 1.0
        if r + 8 < 4 * t - 12:
            vm[t, 64:128] = 1.0
    tri = (np.arange(128)[None, :] >= np.arange(128)[:, None]).astype(
        np.float32
    )
    return wm, vm, tri


def _host_prep(query, key, value, core):
    """Per-core input dict. query/key/value: [B, S, H, D] float32 (full)."""
    import ml_dtypes

    heads = [core + NCORES * i for i in range(HPC)]
    r = (7 - core) % VERT
    q = query[0][:, heads, :]  # [S, 4, D]
    k = key[0][:, heads, :]
    v = value[0][:, heads, :]
    qt = np.ascontiguousarray(q.transpose(1, 2, 0))  # [4, D, S]
    kt = np.ascontiguousarray(k.transpose(1, 2, 0))
    vn = np.ascontiguousarray(v.transpose(1, 0, 2))  # [4, S, D]
    # vertical gather: k-tokens of blocks {r, r+8}
    vtok = np.concatenate([
        np.arange(r * BLOCK, (r + 1) * BLOCK),
        np.arange((r + 8) * BLOCK, (r + 9) * BLOCK),
    ])
    ktv = np.ascontiguousarray(kt[:, :, vtok])  # [4, D, 128]
    vv = np.ascontiguousarray(vn[:, vtok, :])  # [4, 128, D]

    wm, vm, tri = _host_masks(r)

    if MM_DT == "bfloat16":
        dt = ml_dtypes.bfloat16
    else:
        dt = np.float32
    return {
        "qt": qt.astype(dt), "kt": kt.astype(dt), "v": vn.astype(dt),
        "ktv": ktv.astype(dt), "vv": vv.astype(dt),
        "wm": wm.astype(dt), "tri": tri.astype(dt), "vmask": vm,
    }


def kernel(query, key, value, _trace=False, _tmpdir=None):
    """Full-input entry point: [1, 2048, 32, 128] f32 each -> same shape."""
    _install_ntff_shim()
    from concourse.bass_utils import run_bass_kernel_spmd

    query = np.asarray(query, dtype=np.float32)
    key = np.asarray(key, dtype=np.float32)
    value = np.asarray(value, dtype=np.float32)

    nc = _build()
    in_maps = [_host_prep(query, key, value, c) for c in range(NCORES)]
    res = run_bass_kernel_spmd(
        nc, in_maps, core_ids=list(range(NCORES)),
        trace=_trace, tmpdir=_tmpdir,
    )
    out = np.empty((B, S, H, D), dtype=np.float32)
    for c in range(NCORES):
        o = res.results[c]["o"] / res.results[c]["den"]  # [4, D, S] / [4, 1, S]
        for i in range(HPC):
            out[0, :, c + NCORES * i, :] = o[i].T
    kernel.last_result = res
    return out


# revision 13
# speedup vs baseline: 1.1957x; 1.0352x over previous
"""Block-sparse local+vertical-stride causal attention for Trainium2 (Bass/Tile).

Problem: B=1, S=2048, H=32, D=128, sparse_block=64, local_blocks=16,
vert_stride=8, head_sliding_step=1. Mask per head h:
  causal(q,k) AND ( (q_blk - k_blk < 16) OR ((k_blk + h + 1) % 8 == 0) )

Sharding: 8 cores; core c computes heads {c, c+8, c+16, c+24}. All four share
the same vertical-stride residue r = (7 - c) % 8, so a single compiled SPMD
program works for every core with per-core *data* (small masks + pre-gathered
vertical K/V blocks); the code is identical on all cores.

Device algorithm per (head, q-window of 256 tokens = 4 sparse blocks):
  scores S_T[k, q] on the PE in bf16 (same PE rate as fp32r, half the
  DMA/SBUF), tiled to track the sparse structure exactly:
    t >= 4:  L0 (k-blocks 4t-16,4t-15; only qb0 -> 64 q-cols)
             L1 (4t-14,4t-13; qb0..2 -> 192 q-cols)
             L2..L7 (4t-12..4t-1; full 256)
             D0 (4t,4t+1; full 256, causal triangle on chunk 0)
             D1 (4t+2,4t+3; 128 q-cols, causal triangle)
             vert (pre-gathered blocks {r, r+8}; full 256; per-partition
                   0/1 vmask selects blocks with kb < 4t-12)
    t < 4:   2t full L-tiles from block 0 (all-local) + D0 + D1
  One exp per PSUM region on ScalarE (scale folded in; bf16 out). Masking:
    - wm_small [128, 256] multiplicative mask on the L0/L1 region (dead
      sub-blocks + vert/local double-count resolution) - per-window data
    - constant aligned [128,128] causal triangle multiplied into the two
      diag chunks
    - vmask per-partition tensor_scalar on the vert tile
  PV: out_T[d, q] += V.T-stationary @ exp-moving; denominator via a
  ones-column matmul chain riding the same PSUM bank. Host divides.
Host reassembles heads and flips the per-head [d, q] layout to [q, d].
"""

import sys
import types

import numpy as np

# ----------------------------------------------------------------------------
# problem constants (hardcoded per contract; kernel.py must be self-contained)
B, S, H, D = 1, 2048, 32, 128
BLOCK = 64
LOCAL = 16
VERT = 8
NCORES = 8
HPC = H // NCORES  # heads per core (4)
QT = 256  # q tokens per window
NT = S // QT  # 8 windows per head
NKT = S // 128  # 16 k-tiles of 128 tokens per head
SCALE = float(D) ** -0.5

MM_DT = "bfloat16"  # matmul input dtype ("bfloat16" or "float32r")

# psA column layout for t >= 4: [L0 64 | L1 192 | L2..L7 6x256] = 1792
PSA_COLS = 1792
# psB column layout: [D0 256 | D1 128 | pad 128 | vert 256] = 768
# (pad keeps the vert region inside a single PSUM bank: cols 512:768)
PSB_COLS = 768
VOFF = 512


def _install_ntff_shim():
    """bass_utils wants antenv.axon_hooks (absent in this image); provide it,
    backed by the ctypes NTFF profiler from trn_agent_boot when available."""
    if "antenv.axon_hooks" in sys.modules:
        return
    hook = None
    try:
        from trn_agent_boot.trn_boot import _ntff_profile_via_ctypes

        hook = _ntff_profile_via_ctypes("/opt/axon/libaxon_pjrt.so")
    except Exception:
        hook = None
    m = types.ModuleType("antenv.axon_hooks")
    m.get_axon_ntff_profile_hook = lambda: hook
    m.set_axon_ntff_profile_hook = lambda h: None
    sys.modules["antenv.axon_hooks"] = m


_NC = None


def _build():
    """Build + compile the per-core Bass program (one NEFF, all cores)."""
    global _NC
    if _NC is not None:
        return _NC
    import concourse.mybir as mybir
    import concourse.tile as tile
    from concourse import bacc

    f32 = mybir.dt.float32
    mdt = getattr(mybir.dt, MM_DT)

    nc = bacc.Bacc("TRN2", target_bir_lowering=False, debug=False,
                   num_devices=NCORES)

    qt_d = nc.dram_tensor("qt", [HPC, D, S], mdt, kind="ExternalInput")
    kt_d = nc.dram_tensor("kt", [HPC, D, S], mdt, kind="ExternalInput")
    v_d = nc.dram_tensor("v", [HPC, S, D], mdt, kind="ExternalInput")
    ktv_d = nc.dram_tensor("ktv", [HPC, D, 128], mdt, kind="ExternalInput")
    vv_d = nc.dram_tensor("vv", [HPC, 128, D], mdt, kind="ExternalInput")
    wm_d = nc.dram_tensor("wm", [NT - 4, 128, 256], mdt, kind="ExternalInput")
    tri_d = nc.dram_tensor("tri", [128, 128], mdt, kind="ExternalInput")
    vm_d = nc.dram_tensor("vmask", [NT, 128], f32, kind="ExternalInput")
    o_d = nc.dram_tensor("o", [HPC, D, S], f32, kind="ExternalOutput")
    den_d = nc.dram_tensor("den", [HPC, 1, S], f32, kind="ExternalOutput")

    with tile.TileContext(nc) as tc:
        with (
            tc.tile_pool(name="consts", bufs=1) as consts,
            tc.tile_pool(name="io", bufs=2) as io,
            tc.tile_pool(name="exps", bufs=2) as exps,
            tc.tile_pool(name="psA", bufs=1, space="PSUM") as psA,
            tc.tile_pool(name="psB", bufs=1, space="PSUM") as psB,
            tc.tile_pool(name="psPV", bufs=2, space="PSUM") as psPV,
        ):
            tri = consts.tile([128, 128], mdt)
            nc.gpsimd.dma_start(out=tri, in_=tri_d.ap())
            vmask = consts.tile([128, NT], f32)
            nc.gpsimd.dma_start(out=vmask, in_=vm_d.ap().rearrange("t p -> p t"))
            wmask = consts.tile([128, NT - 4, 256], mdt)
            nc.gpsimd.dma_start(out=wmask,
                                in_=wm_d.ap().rearrange("t p q -> p t q"))
            ones_f32 = consts.tile([128, 1], f32)
            nc.vector.memset(ones_f32, 1.0)
            ones_col = consts.tile([128, 1], mdt)
            nc.vector.tensor_copy(out=ones_col, in_=ones_f32)

            def load_head(h, fine):
                """Allocate per-head io tiles and issue their input DMAs.
                fine=True (head 0): window-granular chunks alternating
                between the two HWDGE queues (sync/scalar) so the PE can
                start as soon as the first 128KB lands."""
                qt_t = io.tile([128, S], mdt, tag="qt")
                kt_t = io.tile([128, S], mdt, tag="kt")
                v_t = io.tile([128, NKT, 128], mdt, tag="v")
                ktv_t = io.tile([128, 128], mdt, tag="ktv")
                vv_t = io.tile([128, 128], mdt, tag="vv")
                t = {"qt": qt_t, "kt": kt_t, "v": v_t, "ktv": ktv_t,
                     "vv": vv_t}
                vre = v_d.ap()[h].rearrange("(j p) d -> p j d", p=128)
                if fine:
                    eng = [nc.sync, nc.scalar]
                    # (kt cols, qt cols, v k-tiles) per chunk, matched to
                    # window consumption order
                    chunks = [(256, 256, 2), (512, 512, 4), (1024, 1024, 8),
                              (1536, 1536, 12), (2048, 2048, 16)]
                    pk = pq = pv = 0
                    for i, (ck, cq, cv) in enumerate(chunks):
                        e = eng[i % 2]
                        e.dma_start(out=t["kt"][:, pk:ck],
                                    in_=kt_d.ap()[h][:, pk:ck])
                        e.dma_start(out=t["qt"][:, pq:cq],
                                    in_=qt_d.ap()[h][:, pq:cq])
                        e.dma_start(out=t["v"][:, pv:cv, :],
                                    in_=vre[:, pv:cv, :])
                        pk, pq, pv = ck, cq, cv
                else:
                    nc.sync.dma_start(out=t["kt"][:, 0:512],
                                      in_=kt_d.ap()[h][:, 0:512])
                    nc.sync.dma_start(out=t["qt"][:, 0:512],
                                      in_=qt_d.ap()[h][:, 0:512])
                    nc.sync.dma_start(out=t["v"][:, 0:4, :], in_=vre[:, 0:4, :])
                    nc.sync.dma_start(out=t["kt"][:, 512:2048],
                                      in_=kt_d.ap()[h][:, 512:2048])
                    nc.sync.dma_start(out=t["qt"][:, 512:2048],
                                      in_=qt_d.ap()[h][:, 512:2048])
                    nc.sync.dma_start(out=t["v"][:, 4:, :], in_=vre[:, 4:, :])
                nc.gpsimd.dma_start(out=t["ktv"], in_=ktv_d.ap()[h])
                nc.gpsimd.dma_start(out=t["vv"], in_=vv_d.ap()[h])
                return t

            head_tiles = load_head(0, fine=True)
            for h in range(HPC):
                qt_sb = head_tiles["qt"]
                kt_sb = head_tiles["kt"]
                v_sb = head_tiles["v"]
                ktv_sb = head_tiles["ktv"]
                vv_sb = head_tiles["vv"]

                outT = io.tile([128, S], f32, tag="outT")
                den_sb = io.tile([1, S], f32, tag="den_sb")

                def emit_scores(t):
                    """Scores matmuls + exp + masks for window t. Returns the
                    (exp-slice, v-slice) list for the PV/den stage."""
                    q_sl = qt_sb[:, t * QT:(t + 1) * QT]
                    srcs = []

                    expA = None
                    if t >= 4:
                        sA = psA.tile([128, PSA_COLS], f32, tag="sA")
                        # L0: k-blocks 4t-16,4t-15 (k-tile 2t-8), qb0 only
                        nc.tensor.matmul(
                            sA[:, 0:64],
                            kt_sb[:, (2 * t - 8) * 128:(2 * t - 7) * 128],
                            q_sl[:, 0:64], start=True, stop=True,
                        )
                        # L1: k-blocks 4t-14,4t-13 (k-tile 2t-7), qb0..2
                        nc.tensor.matmul(
                            sA[:, 64:256],
                            kt_sb[:, (2 * t - 7) * 128:(2 * t - 6) * 128],
                            q_sl[:, 0:192], start=True, stop=True,
                        )
                        # L2..L7: k-tiles 2t-6 .. 2t-1, full 256
                        for j in range(6):
                            kt_i = 2 * t - 6 + j
                            nc.tensor.matmul(
                                sA[:, 256 * (j + 1):256 * (j + 2)],
                                kt_sb[:, kt_i * 128:(kt_i + 1) * 128],
                                q_sl, start=True, stop=True,
                            )
                        expA = exps.tile([128, PSA_COLS], mdt, tag="expA")
                        nc.scalar.activation(
                            expA, sA, mybir.ActivationFunctionType.Exp,
                            scale=SCALE,
                        )
                        # data-driven mask on the L0/L1 region (dead blocks,
                        # vert/local double-count)
                        nc.vector.tensor_mul(
                            expA[:, 0:256], expA[:, 0:256], wmask[:, t - 4, :]
                        )
                        # full-width tiles first: the first matmul of the
                        # PV/den accumulation chains must cover the whole
                        # [0:256] q range so has_written is set everywhere
                        for j in range(6):
                            srcs.append((expA[:, 256 * (j + 1):256 * (j + 2)],
                                         v_sb[:, 2 * t - 6 + j, :]))
                        srcs.append((expA[:, 0:64], v_sb[:, 2 * t - 8, :]))
                        srcs.append((expA[:, 64:256], v_sb[:, 2 * t - 7, :]))
                    elif t >= 1:
                        sA = psA.tile([128, 512 * t], f32, tag="sA")
                        for j in range(2 * t):
                            nc.tensor.matmul(
                                sA[:, 256 * j:256 * (j + 1)],
                                kt_sb[:, j * 128:(j + 1) * 128],
                                q_sl, start=True, stop=True,
                            )
                        expA = exps.tile([128, 512 * t], mdt, tag="expA")
                        nc.scalar.activation(
                            expA, sA, mybir.ActivationFunctionType.Exp,
                            scale=SCALE,
                        )
                        for j in range(2 * t):
                            srcs.append((expA[:, 256 * j:256 * (j + 1)],
                                         v_sb[:, j, :]))

                    # B region: D0 full, D1 chunk1 only, vert (t>=4)
                    vert = t >= 4
                    sB = psB.tile([128, PSB_COLS], f32, tag="sB")
                    nc.tensor.matmul(
                        sB[:, 0:256],
                        kt_sb[:, (2 * t) * 128:(2 * t + 1) * 128],
                        q_sl, start=True, stop=True,
                    )
                    nc.tensor.matmul(
                        sB[:, 256:384],
                        kt_sb[:, (2 * t + 1) * 128:(2 * t + 2) * 128],
                        q_sl[:, 128:256], start=True, stop=True,
                    )
                    if vert:
                        nc.tensor.matmul(
                            sB[:, VOFF:VOFF + 256], ktv_sb, q_sl,
                            start=True, stop=True,
                        )
                    expB = exps.tile([128, PSB_COLS], mdt, tag="expB")
                    ncols = VOFF + 256 if vert else 384
                    nc.scalar.activation(
                        expB[:, 0:ncols], sB[:, 0:ncols],
                        mybir.ActivationFunctionType.Exp, scale=SCALE,
                    )
                    # causal triangles on the two diag chunks
                    nc.vector.tensor_mul(expB[:, 0:128], expB[:, 0:128], tri)
                    nc.vector.tensor_mul(expB[:, 256:384], expB[:, 256:384],
                                         tri)
                    if vert:
                        nc.vector.tensor_scalar_mul(
                            out=expB[:, VOFF:VOFF + 256],
                            in0=expB[:, VOFF:VOFF + 256],
                            scalar1=vmask[:, t:t + 1],
                        )
                    srcs.append((expB[:, 0:256], v_sb[:, 2 * t, :]))
                    srcs.append((expB[:, 256:384], v_sb[:, 2 * t + 1, :]))
                    if vert:
                        srcs.append((expB[:, VOFF:VOFF + 256], vv_sb))
                    return srcs

                def emit_pv(t, srcs):
                    """PV + denominator matmul chains, then stage out_T and
                    den to SBUF and DMA out_T."""
                    pv = psPV.tile([128, 512], f32, tag="pv")
                    n_mm = len(srcs)
                    # q-column range of each exp slice within the window:
                    # L0 -> [0:64], L1 -> [0:192], D1 -> [128:256], else full
                    def qrange(e_sl):
                        n = e_sl.shape[-1]
                        if n == 64:
                            return 0, 64
                        if n == 192:
                            return 0, 192
                        if n == 128:
                            return 128, 256
                        return 0, 256

                    for k, (e_sl, v_sl) in enumerate(srcs):
                        lo, hi = qrange(e_sl)
                        nc.tensor.matmul(
                            pv[:, lo:hi], v_sl, e_sl,
                            start=(k == 0), stop=(k == n_mm - 1),
                        )
                    for k, (e_sl, _) in enumerate(srcs):
                        lo, hi = qrange(e_sl)
                        nc.tensor.matmul(
                            pv[0:1, 256 + lo:256 + hi], ones_col, e_sl,
                            start=(k == 0), stop=(k == n_mm - 1),
                        )
                    nc.vector.tensor_copy(
                        out=den_sb[0:1, t * QT:(t + 1) * QT],
                        in_=pv[0:1, 256:512],
                    )
                    nc.vector.tensor_copy(
                        out=outT[:, t * QT:(t + 1) * QT], in_=pv[:, 0:256]
                    )
                    nc.sync.dma_start(
                        out=o_d.ap()[h][:, t * QT:(t + 1) * QT],
                        in_=outT[:, t * QT:(t + 1) * QT],
                    )

                # software-pipelined emission: scores(t+1) lands before pv(t)
                # so the PE always has matmul work while ACT/DVE process t
                prev = None
                for t in range(NT):
                    srcs = emit_scores(t)
                    if prev is not None:
                        emit_pv(prev[0], prev[1])
                        if prev[0] == 3:
                            nc.sync.dma_start(
                                out=den_d.ap()[h][:, 0:1024],
                                in_=den_sb[:, 0:1024],
                            )
                    if t == 4 and h + 1 < HPC:
                        # prefetch the next head's inputs while this head
                        # still has ~4 windows of compute left
                        next_tiles = load_head(h + 1, fine=False)
                    prev = (t, srcs)
                emit_pv(prev[0], prev[1])

                nc.sync.dma_start(out=den_d.ap()[h][:, 1024:2048],
                                  in_=den_sb[:, 1024:2048])
                if h + 1 < HPC:
                    head_tiles = next_tiles

    nc.compile()
    _NC = nc
    return nc


def _host_masks(r):
    """Per-core mask data: wm [NT,128,256], vmask [NT,128], tri [128,128]."""
    # wm: multiplies expA[:, 0:256] (L0 cols 0:64 + L1 cols 64:256), t >= 4.
    # partition p: L0 -> block 4t-16 (p<64) / 4t-15 (p>=64)
    #              L1 -> block 4t-14 (p<64) / 4t-13 (p>=64)
    wm_full = np.ones((NT, 128, 256), dtype=np.float32)
    wm = wm_full  # filled below; rows [4:] shipped to the device
    for t in range(4, NT):
        bl0, bl1 = 4 * t - 16, 4 * t - 15
        bl2, bl3 = 4 * t - 14, 4 * t - 13
        vc = lambda kb: kb % VERT == r  # vert tile covers kb (kb < 4t-12 here)
        # L0 region (cols 0:64 = qb0)
        wm[t, 0:64, 0:64] = 0.0  # block 4t-16 never local
        if vc(bl1):
            wm[t, 64:128, 0:64] = 0.0  # vert tile owns block 4t-15
        # L1 region (cols 64:256 = qb0..2)
        if vc(bl2):
            wm[t, 0:64, 64:256] = 0.0
        else:
            wm[t, 0:64, 64 + 128:256] = 0.0  # qb2 not local for 4t-14
        if vc(bl3):
            wm[t, 64:128, 64:256] = 0.0
        # block 4t-13 local for qb0..2 (all L1 cols) when not vert-covered
    # vmask: vert tile partitions p -> kb = r (p<64) / r+8 (p>=64);
    # active iff kb < 4t-12
    vm = np.zeros((NT, 128), dtype=np.float32)
    for t in range(4, NT):
        if r < 4 * t - 12:
            vm[t, 0:64] = 1.0
        if r + 8 < 4 * t - 12:
            vm[t, 64:128] = 1.0
    tri = (np.arange(128)[None, :] >= np.arange(128)[:, None]).astype(
        np.float32
    )
    return np.ascontiguousarray(wm_full[4:]), vm, tri


def _host_prep(query, key, value, core):
    """Per-core input dict. query/key/value: [B, S, H, D] float32 (full)."""
    import ml_dtypes

    heads = [core + NCORES * i for i in range(HPC)]
    r = (7 - core) % VERT
    q = query[0][:, heads, :]  # [S, 4, D]
    k = key[0][:, heads, :]
    v = value[0][:, heads, :]
    qt = np.ascontiguousarray(q.transpose(1, 2, 0))  # [4, D, S]
    kt = np.ascontiguousarray(k.transpose(1, 2, 0))
    vn = np.ascontiguousarray(v.transpose(1, 0, 2))  # [4, S, D]
    # vertical gather: k-tokens of blocks {r, r+8}
    vtok = np.concatenate([
        np.arange(r * BLOCK, (r + 1) * BLOCK),
        np.arange((r + 8) * BLOCK, (r + 9) * BLOCK),
    ])
    ktv = np.ascontiguousarray(kt[:, :, vtok])  # [4, D, 128]
    vv = np.ascontiguousarray(vn[:, vtok, :])  # [4, 128, D]

    wm, vm, tri = _host_masks(r)

    if MM_DT == "bfloat16":
        dt = ml_dtypes.bfloat16
    else:
        dt = np.float32
    return {
        "qt": qt.astype(dt), "kt": kt.astype(dt), "v": vn.astype(dt),
        "ktv": ktv.astype(dt), "vv": vv.astype(dt),
        "wm": wm.astype(dt), "tri": tri.astype(dt), "vmask": vm,
    }


def kernel(query, key, value, _trace=False, _tmpdir=None):
    """Full-input entry point: [1, 2048, 32, 128] f32 each -> same shape."""
    _install_ntff_shim()
    from concourse.bass_utils import run_bass_kernel_spmd

    query = np.asarray(query, dtype=np.float32)
    key = np.asarray(key, dtype=np.float32)
    value = np.asarray(value, dtype=np.float32)

    nc = _build()
    in_maps = [_host_prep(query, key, value, c) for c in range(NCORES)]
    res = run_bass_kernel_spmd(
        nc, in_maps, core_ids=list(range(NCORES)),
        trace=_trace, tmpdir=_tmpdir,
    )
    out = np.empty((B, S, H, D), dtype=np.float32)
    for c in range(NCORES):
        o = res.results[c]["o"] / res.results[c]["den"]  # [4, D, S] / [4, 1, S]
        for i in range(HPC):
            out[0, :, c + NCORES * i, :] = o[i].T
    kernel.last_result = res
    return out
